# revision 54
# baseline (speedup 1.0000x reference)
"""Trainium2 Bass kernel for nn_Attention_6794638262338.

Single-layer attention block with BitNet-style ternary-quantized projections:
    x -> LN1 -> qkv proj (ternary W) -> MHA softmax -> LN2 -> out proj (ternary W)

Strategy: pure data parallelism. batch=8, n_cores=8 -> one batch element per
core, no collectives. Each core runs an identical Bass/Tile program.

Math folds (host side):
  - ternary_quant(W) = T * s with T in {-1,0,1}: pass T in bf16 (exact), fold
    s_qkv^2 * DIM_HEAD^-0.5 into the exp() activation scale, fold s_qkv/s_out
    into the LN2 rsqrt epsilon/scale.
  - softmax denominator: out = (sum_m exp(s)*v) / colsum. colsum obtained free
    via a ones-column in the attn@v stationary (M=65); division via DVE
    approx-reciprocal + DRAM-bounce partition broadcast + DVE multiply.
  - LN2: mean/var via ones-matmul column sums of a^T / a^T-squared; y =
    (z - mu*W1) * rstd with W1 = rowsum of effective output weight.

Schedule: n is split into two 512-column halves processed half-major, so the
output projection + LN2 of half 0 overlaps with half 1's attention. Odd heads'
attn@v psums sit at partition base 63 so divided outputs land directly on
partitions 64..127 of the pair chunk (no partition-remap DMA).
"""

import numpy as np
from contextlib import ExitStack

import concourse.bass as bass
import concourse.mybir as mybir
import concourse.tile as tile
from concourse import bacc
from concourse.bass import ts, ds
from concourse.bass_utils import run_bass_kernel_spmd
from concourse.masks import make_identity

F32 = mybir.dt.float32
BF16 = mybir.dt.bfloat16
AF = mybir.ActivationFunctionType
ALU = mybir.AluOpType

B, N, D = 8, 1024, 512
H, DH = 8, 64
INNER = H * DH  # 512
NT = N // 128   # 8 n-tiles
DC = D // 128   # 4 d-chunks
NH = N // 2     # half width (512)
EPS_LN = 1e-5
EPS_Q = 1e-6

TRACE = False          # set by test.py to capture an NTFF profile
LAST_RESULTS = None    # BassKernelResults of the most recent run

_CACHE = {}


def _ternary(w):
    """Replicate reference ternary_quant in fp32; return (unit ternary, scale)."""
    w = np.asarray(w, np.float32)
    s = np.float32(np.mean(np.abs(w), dtype=np.float32))
    t = np.round(np.clip(w / (s + np.float32(EPS_Q)), -1.0, 1.0)).astype(np.float32)
    return t, float(s)


def _emit(ctx: ExitStack, tc: "tile.TileContext", io: dict, c: dict, sfx: str = ""):
    nc = tc.nc
    dbg = c.get("debug", False)

    def dump(name, ap):
        if dbg:
            d = nc.dram_tensor(f"dbg_{name}{sfx}", list(ap.shape), ap.dtype, kind="ExternalOutput").ap()
            nc.sync.dma_start(out=d, in_=ap)
    x, tqT, toT, w1u, y = io["x"], io["tqT"], io["toT"], io["w1u"], io["y"]

    need_g1 = c["need_g1"]
    need_b1 = c["need_b1"]
    need_bt = c["need_bt"]
    scale_exp = c["scale_exp"]

    # ---------------- pools ----------------
    const_p = ctx.enter_context(tc.tile_pool(name="const" + sfx, bufs=1))
    xp = ctx.enter_context(tc.tile_pool(name="xp" + sfx, bufs=3))
    lnp = ctx.enter_context(tc.tile_pool(name="lnp" + sfx, bufs=4))
    big = ctx.enter_context(tc.tile_pool(name="big" + sfx, bufs=1))
    attp = ctx.enter_context(tc.tile_pool(name="attp" + sfx, bufs=4))
    smp = ctx.enter_context(tc.tile_pool(name="smp" + sfx, bufs=4))
    outp = ctx.enter_context(tc.tile_pool(name="outp" + sfx, bufs=2))
    # PSUM budget (8 banks): ps_s 2x[128,2,512] = 4, ps_o 2x[128,512] = 2,
    # ps_m 2x[128,512] = 2.
    ps_s = ctx.enter_context(tc.tile_pool(name="ps_s" + sfx, bufs=2, space="PSUM"))
    ps_o = ctx.enter_context(tc.tile_pool(name="ps_o" + sfx, bufs=2, space="PSUM"))
    ps_m = ctx.enter_context(tc.tile_pool(name="ps_m" + sfx, bufs=2, space="PSUM"))

    # ---------------- constants ----------------
    ident = const_p.tile([128, 128], BF16)
    make_identity(nc, ident)
    ones128 = const_p.tile([128, 1], BF16)
    nc.vector.memset(ones128, 1.0)
    # ones row at partition 64 for the K=1 reciprocal-broadcast matmul
    ones_b64 = const_p.tile([128, 64], BF16)
    nc.vector.memset(ones_b64, 1.0)
    eps1 = const_p.tile([128, 1], F32)
    nc.vector.memset(eps1, float(EPS_LN))
    eps2 = const_p.tile([128, 1], F32)
    nc.vector.memset(eps2, c["eps_eff"])
    # warm the ln/exp activation table before first use
    warm = const_p.tile([128, 1], F32)
    nc.scalar.activation(warm, eps1, AF.Ln, bias=eps1)
    nc.scalar.activation(warm, warm, AF.Exp, scale=-0.5)

    # weights arrive on the ACT DGE queue so the x tiles own the SP queue;
    # chunked + emitted late so the x tiles win the DMA engines first
    tq_sb = const_p.tile([128, DC, 3 * INNER], BF16)
    tqTr = tqT.rearrange("(c p) o -> p c o", p=128)

    def load_weights_v():
        nc.sync.dma_start(
            out=tq_sb[:, :, ds(2 * INNER, INNER)], in_=tqTr[:, :, ds(2 * INNER, INNER)]
        )

    def load_weights_qk():
        nc.sync.dma_start(
            out=tq_sb[:, :, 0 : 2 * INNER], in_=tqTr[:, :, 0 : 2 * INNER]
        )

    toT_sb = const_p.tile([128, DC, INNER], BF16)
    toT_lo = const_p.tile([64, INNER], BF16)
    w1b = const_p.tile([128, INNER], F32)

    def load_out_weights():
        toTr = toT.rearrange("(c p) o -> p c o", p=128)
        for dc in range(DC):
            nc.sync.dma_start(out=toT_sb[:, dc, :], in_=toTr[:, dc, :])
        # chunk-3 odd-head rows of the output weight, staged at partition
        # base 0 so the projection can read aTs without the partition remap
        nc.sync.dma_start(out=toT_lo, in_=toTr[ds(64, 64), 3, :])
        nc.sync.dma_start(
            out=w1b,
            in_=bass.AP(tensor=w1u.tensor, offset=w1u.offset, ap=[[0, 128]] + list(w1u.ap)),
        )
    if need_g1:
        g1_ap = io["g1v"]
        g1b = const_p.tile([128, D], F32)
        nc.gpsimd.dma_start(
            out=g1b,
            in_=bass.AP(tensor=g1_ap.tensor, offset=g1_ap.offset, ap=[[0, 128]] + list(g1_ap.ap)),
        )
    if need_b1:
        b1_ap = io["b1v"]
        b1b = const_p.tile([128, D], F32)
        nc.gpsimd.dma_start(
            out=b1b,
            in_=bass.AP(tensor=b1_ap.tensor, offset=b1_ap.offset, ap=[[0, 128]] + list(b1_ap.ap)),
        )
    if need_bt:
        bt_ap = io["btv"]
        btb = const_p.tile([128, INNER], F32)
        nc.gpsimd.dma_start(
            out=btb,
            in_=bass.AP(tensor=bt_ap.tensor, offset=bt_ap.offset, ap=[[0, 128]] + list(bt_ap.ap)),
        )

    # ---------------- persistent big tensors ----------------
    # xln^T: [d, n] bf16 as [128, DC, N]   (partition = d within chunk)
    xlnT = big.tile([128, DC, N], BF16)
    # q^T, k^T head-major: [o, n] as [128, pair, N] (o = pair*128 + p)
    qT = big.tile([128, DC, N], BF16)
    kT = big.tile([128, DC, N], BF16)
    # v row-major with ones column: [128, mt, h, 65] (m = mt*128 + p)
    v_sb = big.tile([128, NT, H, DH + 1], BF16)
    nc.vector.memset(v_sb[:, :, :, DH : DH + 1], 1.0)
    # divided attention outputs, o-major: pair chunk c = heads (2c | 2c+1)
    aT2 = big.tile([128, DC, N], BF16)
    # odd heads' divided output staging (pre partition-remap)
    aTs = big.tile([64, DC, N], BF16)
    # squares of aT2 for the LN2 sum-of-squares (+ pair-3 odd head from aTs)
    sq_sb = big.tile([128, DC, N], BF16)
    sq_lo = big.tile([64, N], BF16)
    # LN2 per-half scalar staging [mu | es | musq | var | sd2 | r2 | r2n] x 4nt
    ln2s = big.tile([128, 7, NT // 2], F32)
    # last-pair reciprocal rows (lane 64) for the PE broadcast
    rc64f = big.tile([65, 512], F32)
    rc64b = big.tile([65, 512], BF16)

    cs_dram = nc.dram_tensor("cs_scratch" + sfx, [H, 2, 512], F32).ap()

    # ================ Phase A: load x, LN1, transpose ================
    # x arrives as two individual tiles then two 3-tile strided groups:
    # tile 0 lands fast (LN1 starts ~2us) without serializing the rest
    xts = []
    for g, cnt in ((0, 1), (1, 1), (2, 3), (5, 3)):
        t = xp.tile([128, cnt, D], F32, name="xt", tag=f"xt{g}", bufs=1)
        nc.sync.dma_start(
            out=t,
            in_=bass.AP(tensor=x.tensor, offset=x.offset + g * 128 * D,
                        ap=[[D, 128], [128 * D, cnt], [1, D]]),
        )
        for i in range(cnt):
            xts.append(t[:, i, :])

    def phase_a(nt):
        xt = xts[nt]
        st6 = lnp.tile([128, 6], F32, name="st6", tag="st6")
        nc.vector.bn_stats(st6, xt)
        mv = lnp.tile([128, 2], F32, name="mv", tag="mv")
        nc.vector.bn_aggr(mv, st6)
        # rstd = exp(-0.5*ln(var+eps)) — keeps ACT on the ln/exp table set
        sd = lnp.tile([128, 1], F32, name="sd", tag="sd")
        nc.scalar.activation(sd, mv[:, 1:2], AF.Ln, bias=eps1)
        rs = lnp.tile([128, 1], F32, name="rs", tag="rs")
        nc.scalar.activation(rs, sd, AF.Exp, scale=-0.5)
        xl = xp.tile([128, D], BF16, name="xl", tag="xl")
        if need_g1 or need_b1:
            xlf = xp.tile([128, D], F32, name="xlf", tag="xlf")
            nc.vector.tensor_scalar(
                out=xlf, in0=xt, scalar1=mv[:, 0:1], scalar2=rs,
                op0=ALU.subtract, op1=ALU.mult,
            )
            if need_g1:
                nc.vector.tensor_mul(xlf, xlf, g1b)
            if need_b1:
                nc.vector.tensor_add(xlf, xlf, b1b)
            nc.vector.tensor_copy(xl, xlf)
        else:
            # xl = xt*rs - mu*rs in one ACT pass (per-partition scale/bias)
            nmr = lnp.tile([128, 1], F32, name="nmr", tag="nmr")
            nc.vector.tensor_scalar(
                out=nmr, in0=mv[:, 0:1], scalar1=rs, scalar2=-1.0,
                op0=ALU.mult, op1=ALU.mult,
            )
            nc.scalar.activation(xl, xt, AF.Identity, bias=nmr, scale=rs)
        # transpose via matmul with identity; one psum tile for all 4 chunks
        pt = ps_m.tile([128, DC, 128], F32, name="pt", tag="mm")
        for dc in range(DC):
            nc.tensor.matmul(
                pt[:, dc, :], lhsT=xl[:, ts(dc, 128)], rhs=ident, start=True, stop=True
            )
        # alternate the psum->sbuf copy between DVE and ACT
        if nt % 2 == 0:
            nc.vector.tensor_copy(out=xlnT[:, :, ts(nt, 128)], in_=pt)
        else:
            nc.scalar.copy(out=xlnT[:, :, ts(nt, 128)], in_=pt)

    # ================ qkv projection ================
    def emit_qk(ot, nns=(0, 1)):
        # q, k head-major: psum[o_tile, n] = sum_dc Tq[:,dc,ot].T @ xlnT[:,dc,n]
        for sec, dst in ((0, qT), (1, kT)):
            for nn in nns:
                pq = ps_m.tile([128, 512], F32, name="pq", tag="mm")
                for dc in range(DC):
                    nc.tensor.matmul(
                        pq,
                        lhsT=tq_sb[:, dc, ds(sec * INNER + ot * 128, 128)],
                        rhs=xlnT[:, dc, ts(nn, 512)],
                        start=(dc == 0), stop=(dc == DC - 1),
                    )
                nc.vector.tensor_copy(out=dst[:, ot, ts(nn, 512)], in_=pq)

    def emit_v():
        # v row-major: psum[m_tile, o] = sum_dc xlnT[:,dc,mt].T @ Tq_v[:,dc,:]
        for mt in range(NT):
            pv = ps_m.tile([128, 512], F32, name="pv", tag="mm")
            for dc in range(DC):
                nc.tensor.matmul(
                    pv,
                    lhsT=xlnT[:, dc, ts(mt, 128)],
                    rhs=tq_sb[:, dc, ds(2 * INNER, INNER)],
                    start=(dc == 0), stop=(dc == DC - 1),
                )
            nc.vector.tensor_copy(
                out=v_sb[:, mt, :, 0:DH],
                in_=pv.rearrange("p (h d) -> p h d", h=H),
            )

    # ================ scores + exp (one n-half, one head pair) ================
    def scores_exp(half, pair):
        """Returns [atn_sub0, atn_sub1], each [128, NT, 512] bf16 for this
        half's columns. Scores psums cover two m-tiles -> [128,1024] exps."""
        atns = [
            attp.tile([128, NT, NH], BF16, name=f"atn{s}", tag=f"atn{s}")
            for s in range(2)
        ]
        for mtp in range(NT // 2):
            pss = [
                ps_s.tile([128, 2, 512], F32, name="pssa", tag="s"),
                ps_s.tile([128, 2, 512], F32, name="pssb", tag="s"),
            ]
            for j in range(2):
                mt = 2 * mtp + j
                for sub in range(2):
                    base = sub * 64
                    nc.tensor.matmul(
                        pss[sub][:, j, :],
                        lhsT=kT[ds(base, 64), pair, ts(mt, 128)],
                        rhs=qT[ds(base, 64), pair, ds(half * NH, NH)],
                        start=True, stop=True,
                    )
            for sub in range(2):
                nc.scalar.activation(
                    out=atns[sub][:, 2 * mtp : 2 * mtp + 2, :], in_=pss[sub],
                    func=AF.Exp, scale=scale_exp,
                )
        return atns

    # ================ attn@v + divide for one (half, pair) ================
    def po_div(half, pair, atns):
        for sub in range(2):
            h = 2 * pair + sub
            atn = atns[sub]
            po = ps_o.tile([65, 512], F32, name="po", tag="po")
            for mt in range(NT):
                nc.tensor.matmul(
                    po,
                    lhsT=v_sb[:, mt, h, :],
                    rhs=atn[:, mt, :],
                    start=(mt == 0), stop=(mt == NT - 1),
                )
            # stage numerators + denominator row to SBUF so the psum bank
            # frees fast
            stg = smp.tile([65, 512], F32, name="stg", tag="stg")
            nc.vector.tensor_copy(stg, po)
            # even head lands on partitions 0:64 of the pair chunk directly;
            # odd head stages in aTs then partition-remaps via SBUF->SBUF DMA
            div_dst = (
                aT2[ds(0, 64), pair, ds(half * NH, NH)]
                if sub == 0
                else aTs[:, pair, ds(half * NH, NH)]
            )
            if pair < 3:
                # bounce the raw denominator row through DRAM to broadcast
                # across partitions, then approx-reciprocal (base 0 only);
                # the multiply runs on idle GPSIMD to unload DVE
                nc.sync.dma_start(out=cs_dram[h, half, :], in_=stg[64:65, :])
                rbf = smp.tile([64, 512], F32, name="rbf", tag="rbf")
                src = cs_dram[h, half, :]
                nc.sync.dma_start(
                    out=rbf,
                    in_=bass.AP(tensor=src.tensor, offset=src.offset,
                                ap=[[0, 64]] + list(src.ap)),
                )
                rcp = smp.tile([64, 512], F32, name="rcp", tag="rcp")
                nc.vector.reciprocal_approx_fast(rcp, rbf)
                nc.vector.tensor_tensor(
                    out=div_dst, in0=stg[0:DH, :], in1=rcp, op=ALU.mult,
                )
            else:
                # last pair feeds the LN2/projection tail: skip the DMA
                # round-trip. Exact reciprocal on the psum row (lane 64),
                # bf16 it, then a K=1 ones matmul broadcasts it across
                # partitions into psum; the multiply reads both psums.
                nc.vector.reciprocal(rc64f[64:65, :], po[64:65, :])
                nc.vector.tensor_copy(rc64b[64:65, :], rc64f[64:65, :])
                pb = ps_o.tile([64, 512], F32, name="pb", tag="po")
                nc.tensor.matmul(
                    pb, lhsT=ones_b64[64:65, :], rhs=rc64b[64:65, :],
                    start=True, stop=True,
                )
                nc.vector.tensor_tensor(
                    out=div_dst, in0=stg[0:DH, :], in1=pb, op=ALU.mult,
                )
            if h == 0 and half == 0:
                dump("stg_h0", stg)
                dump("atn_h0", atn)
        if pair < 3:
            nc.sync.dma_start(
                out=aT2[ds(64, 64), pair, ds(half * NH, NH)],
                in_=aTs[:, pair, ds(half * NH, NH)],
            )
            # squares for the LN2 sum-of-squares on idle GPSIMD
            nc.gpsimd.tensor_tensor(
                out=sq_sb[:, pair, ds(half * NH, NH)],
                in0=aT2[:, pair, ds(half * NH, NH)],
                in1=aT2[:, pair, ds(half * NH, NH)],
                op=ALU.mult,
            )
        else:
            # pair 3 feeds the tail: no remap — the projection reads aTs
            # directly. Squares on DVE (tail-critical).
            ev = aT2[ds(0, 64), 3, ds(half * NH, NH)]
            od = aTs[:, 3, ds(half * NH, NH)]
            nc.vector.tensor_tensor(
                out=sq_sb[ds(0, 64), 3, ds(half * NH, NH)], in0=ev, in1=ev,
                op=ALU.mult,
            )
            nc.vector.tensor_tensor(
                out=sq_lo[:, ds(half * NH, NH)], in0=od, in1=od, op=ALU.mult,
            )

    # ================ LN2 stats + scalars for one half ================
    def ln2_stats(half):
        hnt = NT // 2
        s12 = ps_o.tile([128, 2, hnt], F32, name="s12", tag="po")
        o64 = ones128[0:64, :]
        for i in range(hnt):
            nt = half * hnt + i
            for ch in range(3):
                nc.tensor.matmul(
                    s12[:, 0, i : i + 1], lhsT=aT2[:, ch, ts(nt, 128)], rhs=ones128,
                    start=(ch == 0), stop=False,
                )
            nc.tensor.matmul(
                s12[:, 0, i : i + 1], lhsT=aT2[ds(0, 64), 3, ts(nt, 128)],
                rhs=o64, start=False, stop=False,
            )
            nc.tensor.matmul(
                s12[:, 0, i : i + 1], lhsT=aTs[:, 3, ts(nt, 128)],
                rhs=o64, start=False, stop=True,
            )
            for ch in range(3):
                nc.tensor.matmul(
                    s12[:, 1, i : i + 1], lhsT=sq_sb[:, ch, ts(nt, 128)], rhs=ones128,
                    start=(ch == 0), stop=False,
                )
            nc.tensor.matmul(
                s12[:, 1, i : i + 1], lhsT=sq_sb[ds(0, 64), 3, ts(nt, 128)],
                rhs=o64, start=False, stop=False,
            )
            nc.tensor.matmul(
                s12[:, 1, i : i + 1], lhsT=sq_lo[:, ts(nt, 128)],
                rhs=o64, start=False, stop=True,
            )
        # mu = s1/512 ; var = s2/512 - mu^2 ; r2 = s_o / sqrt(var + eps_eff)
        mu, musq, var, sd2, r2, r2n = (ln2s[:, i, :] for i in range(6))
        nc.scalar.mul(mu, s12[:, 0, :], 1.0 / INNER)
        nc.vector.tensor_mul(musq, mu, mu)
        nc.vector.scalar_tensor_tensor(
            out=var, in0=s12[:, 1, :], scalar=1.0 / INNER, in1=musq,
            op0=ALU.mult, op1=ALU.subtract,
        )
        nc.scalar.activation(sd2, var, AF.Ln, bias=eps2, scale=c["inv_so2"])
        nc.scalar.activation(r2, sd2, AF.Exp, scale=-0.5)
        nc.vector.tensor_scalar_mul(r2n, r2, -1.0)

    # ================ output projection + LN2 apply for one n-tile ========
    def zy(nt):
        i = nt % (NT // 2)
        mu = ln2s[:, 0, i : i + 1]
        r2n = ln2s[:, 5, i : i + 1]
        pz = ps_m.tile([128, INNER], F32, name="pz", tag="mm")
        for ch in range(3):
            nc.tensor.matmul(
                pz, lhsT=aT2[:, ch, ts(nt, 128)], rhs=toT_sb[:, ch, :],
                start=(ch == 0), stop=False,
            )
        nc.tensor.matmul(
            pz, lhsT=aT2[ds(0, 64), 3, ts(nt, 128)], rhs=toT_sb[ds(0, 64), 3, :],
            start=False, stop=False,
        )
        nc.tensor.matmul(
            pz, lhsT=aTs[:, 3, ts(nt, 128)], rhs=toT_lo,
            start=False, stop=True,
        )
        # u = (W1*mu) - z straight off psum; y = u * (-r2) on ACT
        ut = outp.tile([128, INNER], F32, name="ut", tag="ut")
        nc.vector.scalar_tensor_tensor(
            out=ut, in0=w1b, scalar=mu, in1=pz,
            op0=ALU.mult, op1=ALU.subtract,
        )
        yt = outp.tile([128, INNER], F32, name="yt", tag="yt")
        # half-0 tiles overlap half-1 attention (DVE has slack there); the
        # final half runs after the exps, when ACT is free
        if nt < 4:
            nc.vector.tensor_scalar_mul(yt, ut, r2n)
        else:
            nc.scalar.mul(yt, ut, r2n)
        if need_bt:
            nc.vector.tensor_add(yt, yt, btb)
        q = nc.scalar if nt == NT - 1 else nc.sync
        q.dma_start(out=y[ts(nt, 128), :], in_=yt)

    # ================ driver ================
    load_weights_qk()
    load_weights_v()
    for nt in range(NT):
        phase_a(nt)
        if nt == 5:
            # first-half xlnT (tiles 0-3) is ready: start pair-0 q/k early
            emit_qk(0, nns=(0,))
    emit_qk(0, nns=(1,))
    emit_qk(1)
    a00 = scores_exp(0, 0)
    a01 = scores_exp(0, 1)
    emit_v()
    load_out_weights()
    emit_qk(2)
    po_div(0, 0, a00)
    a02 = scores_exp(0, 2)
    emit_qk(3)
    po_div(0, 1, a01)
    a03 = scores_exp(0, 3)
    po_div(0, 2, a02)
    po_div(0, 3, a03)
    dump("xlnT", xlnT)
    dump("qT", qT)
    dump("kT", kT)
    dump("v_sb", v_sb)
    dump("aT2", aT2)
    dump("sq_sb", sq_sb)
    ln2_stats(0)
    dump("ln2s", ln2s)
    # half 1 attention overlaps half 0's output projection
    a10 = scores_exp(1, 0)
    zy(0)
    zy(1)
    a11 = scores_exp(1, 1)
    po_div(1, 0, a10)
    zy(2)
    zy(3)
    a12 = scores_exp(1, 2)
    po_div(1, 1, a11)
    a13 = scores_exp(1, 3)
    po_div(1, 2, a12)
    po_div(1, 3, a13)
    ln2_stats(1)
    for nt in range(4, 8):
        zy(nt)


def _build(c: dict):
    nc = bacc.Bacc("TRN2", target_bir_lowering=False, debug=False, num_devices=B)
    io = {
        "x": nc.dram_tensor("x", [N, D], F32, kind="ExternalInput").ap(),
        "tqT": nc.dram_tensor("tqT", [D, 3 * INNER], BF16, kind="ExternalInput").ap(),
        "toT": nc.dram_tensor("toT", [INNER, INNER], BF16, kind="ExternalInput").ap(),
        "w1u": nc.dram_tensor("w1u", [INNER], F32, kind="ExternalInput").ap(),
        "y": nc.dram_tensor("y", [N, D], F32, kind="ExternalOutput").ap(),
    }
    if c["need_g1"]:
        io["g1v"] = nc.dram_tensor("g1v", [D], F32, kind="ExternalInput").ap()
    if c["need_b1"]:
        io["b1v"] = nc.dram_tensor("b1v", [D], F32, kind="ExternalInput").ap()
    if c["need_bt"]:
        io["btv"] = nc.dram_tensor("btv", [INNER], F32, kind="ExternalInput").ap()
    reps = c.get("body_reps", 1)
    with tile.TileContext(nc) as tc:
        for r in range(reps):
            with ExitStack() as ctx:
                _emit(ctx, tc, io, c, sfx="" if r == 0 else f"_r{r}")

    nc.compile()

    # The act-table-load pass greedily picks the first set containing each
    # function, thrashing between `natural_log` (Ln) and `exp_and_others`
    # (Exp) on every rstd computation. All activation funcs this kernel uses
    # (Ln, Exp, Copy, Identity) live together in `natural_log_exp_and_others`,
    # so rewrite the first load to that set and drop the rest.
    from concourse.hw_specs import get_activation_tables
    tset = list(get_activation_tables(nc.m.arch).keys())
    nle = tset.index("natural_log_exp_and_others")
    for blk in nc.main_func.blocks:
        keep, first = [], False
        for inst in blk.instructions:
            if type(inst).__name__ == "InstLoadActFuncSet":
                si = getattr(inst, "sync_info", None)
                clean = si is None or (not si.on_wait and not si.on_update)
                if not first:
                    inst.act_func_set_id = nle
                    first = True
                    keep.append(inst)
                elif not clean:
                    inst.act_func_set_id = nle
                    keep.append(inst)
            else:
                keep.append(inst)
        blk.instructions[:] = keep
    return nc


def _prep(inputs):
    g1 = np.asarray(inputs["g1"], np.float32)
    b1 = np.asarray(inputs["b1"], np.float32)
    g2 = np.asarray(inputs["g2"], np.float32)
    b2 = np.asarray(inputs["b2"], np.float32)
    b_out = np.asarray(inputs["b_out"], np.float32)

    Tq, s_q = _ternary(inputs["W_qkv"])   # [3*inner, d]
    To, s_o = _ternary(inputs["W_out"])   # [dout, o]

    Wp = To * g2[None, :]                 # fold g2 (exact when g2 == 1)
    toT = np.ascontiguousarray(Wp.T)      # [o, dout]
    w1u = Wp.sum(axis=1).astype(np.float32)
    bias_total = (b2 @ To.T) * np.float32(s_o) + b_out

    c = {
        "scale_exp": float(s_q * s_q * (DH ** -0.5)),
        "inv_so2": float(1.0 / (s_o * s_o)),
        "eps_eff": float(EPS_LN / (s_q * s_q * s_o * s_o)),
        "need_g1": bool(not np.allclose(g1, 1.0)),
        "need_b1": bool(np.any(b1)),
        "need_bt": bool(np.any(bias_total)),
    }
    arrs = {
        "tqT": np.ascontiguousarray(Tq.T),
        "toT": toT,
        "w1u": w1u,
        "g1": g1, "b1": b1, "bt": bias_total,
    }
    return c, arrs


def _to_bf16(a):
    import ml_dtypes
    return np.asarray(a, np.float32).astype(ml_dtypes.bfloat16)


def kernel(**inputs) -> np.ndarray:
    global LAST_RESULTS
    x = np.asarray(inputs["x"], np.float32)
    assert x.shape == (B, N, D)
    c, arrs = _prep(inputs)

    key = tuple(sorted(c.items()))
    if key not in _CACHE:
        _CACHE[key] = _build(c)
    nc = _CACHE[key]

    base = {
        "tqT": _to_bf16(arrs["tqT"]),
        "toT": _to_bf16(arrs["toT"]),
        "w1u": arrs["w1u"].astype(np.float32),
    }
    if c["need_g1"]:
        base["g1v"] = arrs["g1"]
    if c["need_b1"]:
        base["b1v"] = arrs["b1"]
    if c["need_bt"]:
        base["btv"] = arrs["bt"].astype(np.float32)

    in_maps = [dict(base, x=np.ascontiguousarray(x[i])) for i in range(B)]
    res = run_bass_kernel_spmd(nc, in_maps, core_ids=list(range(B)), trace=TRACE)
    LAST_RESULTS = res
    out = np.stack([res.results[i]["y"] for i in range(B)], axis=0)
    return out.astype(np.float32)


def bench_exec_ns(inputs, iters=32, reps=5, body_reps=1):
    """Measure per-execution NEFF time by chaining `iters` sequential
    executions inside one jitted program (chained through the output
    buffers) and comparing against a 1-execution program."""
    import time as _time
    import jax
    from jax.experimental.shard_map import shard_map
    from jax.sharding import Mesh, PartitionSpec, NamedSharding
    from concourse import bass2jax, mybir as _mybir

    x = np.asarray(inputs["x"], np.float32)
    c, arrs = _prep(inputs)
    if body_reps != 1:
        c["body_reps"] = body_reps
    key = tuple(sorted(c.items()))
    if key not in _CACHE:
        _CACHE[key] = _build(c)
    nc = _CACHE[key]
    bass2jax.install_neuronx_cc_hook()

    base = {
        "tqT": _to_bf16(arrs["tqT"]),
        "toT": _to_bf16(arrs["toT"]),
        "w1u": arrs["w1u"].astype(np.float32),
    }
    if c["need_g1"]:
        base["g1v"] = arrs["g1"]
    if c["need_b1"]:
        base["b1v"] = arrs["b1"]
    if c["need_bt"]:
        base["btv"] = arrs["bt"].astype(np.float32)
    in_maps = [dict(base, x=np.ascontiguousarray(x[i])) for i in range(B)]

    partition_name = nc.partition_id_tensor.name if nc.partition_id_tensor else None
    in_names, out_names, out_avals, zero_outs = [], [], [], []
    for alloc in nc.m.functions[0].allocations:
        if not isinstance(alloc, mybir.MemoryLocationSet):
            continue
        name = alloc.memorylocations[0].name
        if alloc.kind == "ExternalInput":
            if name != partition_name:
                in_names.append(name)
        elif alloc.kind == "ExternalOutput":
            out_names.append(name)
            shape = tuple(alloc.tensor_shape)
            dtype = mybir.dt.np(alloc.dtype)
            out_avals.append(jax.core.ShapedArray(shape, dtype))
            zero_outs.append(np.zeros(shape, dtype))
    n_params = len(in_names)

    bind_names = list(in_names) + list(out_names)
    if partition_name is not None:
        bind_names.append(partition_name)

    def _body(*args):
        operands = list(args)
        pid = [bass2jax.partition_id_tensor()] if partition_name else []
        outs = bass2jax._bass_exec_p.bind(
            *(operands + pid),
            out_avals=tuple(out_avals),
            in_names=tuple(bind_names),
            out_names=tuple(out_names),
            lowering_input_output_aliases=(),
            sim_require_finite=True,
            sim_require_nnan=True,
            nc=nc,
        )
        return tuple(outs)

    devices = jax.devices()[:B]
    mesh = Mesh(np.asarray(devices), ("core",))
    spec = PartitionSpec("core")
    n_out = len(out_names)
    per_core = [[np.asarray(m[nm]) for nm in in_names] for m in in_maps]
    concat_in = [
        np.concatenate([per_core[cc][i] for cc in range(B)], axis=0)
        for i in range(n_params)
    ]
    concat_zeros = [
        np.zeros((B * z.shape[0], *z.shape[1:]), z.dtype) for z in zero_outs
    ]
    dev_args = [
        jax.device_put(a, NamedSharding(mesh, spec)) for a in concat_in + concat_zeros
    ]

    f = jax.jit(
        shard_map(
            _body, mesh=mesh,
            in_specs=(spec,) * (n_params + n_out),
            out_specs=(spec,) * n_out,
            check_rep=False,
        )
    )
    jax.block_until_ready(f(*dev_args))  # compile + warm

    times = {}
    for k in (1, iters):
        best = float("inf")
        for _ in range(reps):
            t0 = _time.perf_counter()
            r = None
            for _ in range(k):
                r = f(*dev_args)  # async dispatch; device executes in-order
            jax.block_until_ready(r)
            best = min(best, _time.perf_counter() - t0)
        times[k] = best
    exec_ns = (times[iters] - times[1]) / (iters - 1) * 1e9
    return exec_ns, times


# revision 55
# speedup vs baseline: 2.1686x; 2.1686x over previous
"""Trainium2 Bass kernel for nn_Attention_6794638262338.

Single-layer attention block with BitNet-style ternary-quantized projections:
    x -> LN1 -> qkv proj (ternary W) -> MHA softmax -> LN2 -> out proj (ternary W)

Strategy: pure data parallelism. batch=8, n_cores=8 -> one batch element per
core, no collectives. Each core runs an identical Bass/Tile program.

Math folds (host side):
  - ternary_quant(W) = T * s with T in {-1,0,1}: pass T in bf16 (exact), fold
    s_qkv^2 * DIM_HEAD^-0.5 into the exp() activation scale, fold s_qkv/s_out
    into the LN2 rsqrt epsilon/scale.
  - softmax denominator: out = (sum_m exp(s)*v) / colsum. colsum obtained free
    via a ones-column in the attn@v stationary (M=65); division via DVE
    approx-reciprocal + DRAM-bounce partition broadcast + DVE multiply.
  - LN2: mean/var via ones-matmul column sums of a^T / a^T-squared; y =
    (z - mu*W1) * rstd with W1 = rowsum of effective output weight.

Schedule: n is split into two 512-column halves processed half-major, so the
output projection + LN2 of half 0 overlaps with half 1's attention. Odd heads'
attn@v psums sit at partition base 63 so divided outputs land directly on
partitions 64..127 of the pair chunk (no partition-remap DMA).
"""

import numpy as np
from contextlib import ExitStack

import concourse.bass as bass
import concourse.mybir as mybir
import concourse.tile as tile
from concourse import bacc
from concourse.bass import ts, ds
from concourse.bass_utils import run_bass_kernel_spmd
from concourse.masks import make_identity

F32 = mybir.dt.float32
BF16 = mybir.dt.bfloat16
AF = mybir.ActivationFunctionType
ALU = mybir.AluOpType

B, N, D = 8, 1024, 512
H, DH = 8, 64
INNER = H * DH  # 512
NT = N // 128   # 8 n-tiles
DC = D // 128   # 4 d-chunks
NH = N // 2     # half width (512)
EPS_LN = 1e-5
EPS_Q = 1e-6

TRACE = False          # set by test.py to capture an NTFF profile
LAST_RESULTS = None    # BassKernelResults of the most recent run

_CACHE = {}


def _ternary(w):
    """Replicate reference ternary_quant in fp32; return (unit ternary, scale)."""
    w = np.asarray(w, np.float32)
    s = np.float32(np.mean(np.abs(w), dtype=np.float32))
    t = np.round(np.clip(w / (s + np.float32(EPS_Q)), -1.0, 1.0)).astype(np.float32)
    return t, float(s)


def _emit(ctx: ExitStack, tc: "tile.TileContext", io: dict, c: dict, sfx: str = ""):
    nc = tc.nc
    dbg = c.get("debug", False)

    def dump(name, ap):
        if dbg:
            d = nc.dram_tensor(f"dbg_{name}{sfx}", list(ap.shape), ap.dtype, kind="ExternalOutput").ap()
            nc.sync.dma_start(out=d, in_=ap)
    x, tqT, toT, w1u, y = io["x"], io["tqT"], io["toT"], io["w1u"], io["y"]

    need_g1 = c["need_g1"]
    need_b1 = c["need_b1"]
    need_bt = c["need_bt"]
    scale_exp = c["scale_exp"]

    # ---------------- pools ----------------
    const_p = ctx.enter_context(tc.tile_pool(name="const" + sfx, bufs=1))
    xp = ctx.enter_context(tc.tile_pool(name="xp" + sfx, bufs=3))
    lnp = ctx.enter_context(tc.tile_pool(name="lnp" + sfx, bufs=4))
    big = ctx.enter_context(tc.tile_pool(name="big" + sfx, bufs=1))
    attp = ctx.enter_context(tc.tile_pool(name="attp" + sfx, bufs=4))
    smp = ctx.enter_context(tc.tile_pool(name="smp" + sfx, bufs=4))
    outp = ctx.enter_context(tc.tile_pool(name="outp" + sfx, bufs=2))
    # PSUM budget (8 banks): ps_s 2x[128,2,512] = 4, ps_o 2x[128,512] = 2,
    # ps_m 2x[128,512] = 2.
    ps_s = ctx.enter_context(tc.tile_pool(name="ps_s" + sfx, bufs=2, space="PSUM"))
    ps_o = ctx.enter_context(tc.tile_pool(name="ps_o" + sfx, bufs=2, space="PSUM"))
    ps_m = ctx.enter_context(tc.tile_pool(name="ps_m" + sfx, bufs=2, space="PSUM"))

    # ---------------- constants ----------------
    ident = const_p.tile([128, 128], BF16)
    make_identity(nc, ident)
    ones128 = const_p.tile([128, 1], BF16)
    nc.vector.memset(ones128, 1.0)
    # ones row at partition 64 for the K=1 reciprocal-broadcast matmul
    ones_b64 = const_p.tile([128, 64], BF16)
    nc.vector.memset(ones_b64, 1.0)
    eps1 = const_p.tile([128, 1], F32)
    nc.vector.memset(eps1, float(EPS_LN))
    eps2 = const_p.tile([128, 1], F32)
    nc.vector.memset(eps2, c["eps_eff"])
    # warm the ln/exp activation table before first use
    warm = const_p.tile([128, 1], F32)
    nc.scalar.activation(warm, eps1, AF.Ln, bias=eps1)
    nc.scalar.activation(warm, warm, AF.Exp, scale=-0.5)

    # weights arrive on the ACT DGE queue so the x tiles own the SP queue;
    # chunked + emitted late so the x tiles win the DMA engines first
    tq_sb = const_p.tile([128, DC, 3 * INNER], BF16)
    tqTr = tqT.rearrange("(c p) o -> p c o", p=128)

    def load_weights_v():
        nc.sync.dma_start(
            out=tq_sb[:, :, ds(2 * INNER, INNER)], in_=tqTr[:, :, ds(2 * INNER, INNER)]
        )

    def load_weights_qk():
        nc.sync.dma_start(
            out=tq_sb[:, :, 0 : 2 * INNER], in_=tqTr[:, :, 0 : 2 * INNER]
        )

    toT_sb = const_p.tile([128, DC, INNER], BF16)
    toT_lo = const_p.tile([64, INNER], BF16)
    w1b = const_p.tile([128, INNER], F32)

    def load_out_weights():
        toTr = toT.rearrange("(c p) o -> p c o", p=128)
        for dc in range(DC):
            nc.sync.dma_start(out=toT_sb[:, dc, :], in_=toTr[:, dc, :])
        # chunk-3 odd-head rows of the output weight, staged at partition
        # base 0 so the projection can read aTs without the partition remap
        nc.sync.dma_start(out=toT_lo, in_=toTr[ds(64, 64), 3, :])
        nc.sync.dma_start(
            out=w1b,
            in_=bass.AP(tensor=w1u.tensor, offset=w1u.offset, ap=[[0, 128]] + list(w1u.ap)),
        )
    if need_g1:
        g1_ap = io["g1v"]
        g1b = const_p.tile([128, D], F32)
        nc.gpsimd.dma_start(
            out=g1b,
            in_=bass.AP(tensor=g1_ap.tensor, offset=g1_ap.offset, ap=[[0, 128]] + list(g1_ap.ap)),
        )
    if need_b1:
        b1_ap = io["b1v"]
        b1b = const_p.tile([128, D], F32)
        nc.gpsimd.dma_start(
            out=b1b,
            in_=bass.AP(tensor=b1_ap.tensor, offset=b1_ap.offset, ap=[[0, 128]] + list(b1_ap.ap)),
        )
    if need_bt:
        bt_ap = io["btv"]
        btb = const_p.tile([128, INNER], F32)
        nc.gpsimd.dma_start(
            out=btb,
            in_=bass.AP(tensor=bt_ap.tensor, offset=bt_ap.offset, ap=[[0, 128]] + list(bt_ap.ap)),
        )

    # ---------------- persistent big tensors ----------------
    # xln^T: [d, n] bf16 as [128, DC, N]   (partition = d within chunk)
    xlnT = big.tile([128, DC, N], BF16)
    # q^T, k^T head-major: [o, n] as [128, pair, N] (o = pair*128 + p)
    qT = big.tile([128, DC, N], BF16)
    kT = big.tile([128, DC, N], BF16)
    # v row-major with ones column: [128, mt, h, 65] (m = mt*128 + p)
    v_sb = big.tile([128, NT, H, DH + 1], BF16)
    nc.vector.memset(v_sb[:, :, :, DH : DH + 1], 1.0)
    # divided attention outputs, o-major: pair chunk c = heads (2c | 2c+1)
    aT2 = big.tile([128, DC, N], BF16)
    # odd heads' divided output staging (pre partition-remap)
    aTs = big.tile([64, DC, N], BF16)
    # squares of aT2 for the LN2 sum-of-squares (+ pair-3 odd head from aTs)
    sq_sb = big.tile([128, DC, N], BF16)
    sq_lo = big.tile([64, N], BF16)
    # LN2 per-half scalar staging [mu | es | musq | var | sd2 | r2 | r2n] x 4nt
    ln2s = big.tile([128, 7, NT // 2], F32)
    # last-pair reciprocal rows (lane 64) for the PE broadcast
    rc64f = big.tile([65, 512], F32)
    rc64b = big.tile([65, 512], BF16)

    cs_dram = nc.dram_tensor("cs_scratch" + sfx, [H, 2, 512], F32).ap()

    # ================ Phase A: load x, LN1, transpose ================
    # x arrives as two individual tiles then two 3-tile strided groups:
    # tile 0 lands fast (LN1 starts ~2us) without serializing the rest
    xts = []
    for g, cnt in ((0, 1), (1, 1), (2, 3), (5, 3)):
        t = xp.tile([128, cnt, D], F32, name="xt", tag=f"xt{g}", bufs=1)
        nc.sync.dma_start(
            out=t,
            in_=bass.AP(tensor=x.tensor, offset=x.offset + g * 128 * D,
                        ap=[[D, 128], [128 * D, cnt], [1, D]]),
        )
        for i in range(cnt):
            xts.append(t[:, i, :])

    def phase_a(nt):
        xt = xts[nt]
        st6 = lnp.tile([128, 6], F32, name="st6", tag="st6")
        nc.vector.bn_stats(st6, xt)
        mv = lnp.tile([128, 2], F32, name="mv", tag="mv")
        nc.vector.bn_aggr(mv, st6)
        # rstd = exp(-0.5*ln(var+eps)) — keeps ACT on the ln/exp table set
        sd = lnp.tile([128, 1], F32, name="sd", tag="sd")
        nc.scalar.activation(sd, mv[:, 1:2], AF.Ln, bias=eps1)
        rs = lnp.tile([128, 1], F32, name="rs", tag="rs")
        nc.scalar.activation(rs, sd, AF.Exp, scale=-0.5)
        xl = xp.tile([128, D], BF16, name="xl", tag="xl")
        if need_g1 or need_b1:
            xlf = xp.tile([128, D], F32, name="xlf", tag="xlf")
            nc.vector.tensor_scalar(
                out=xlf, in0=xt, scalar1=mv[:, 0:1], scalar2=rs,
                op0=ALU.subtract, op1=ALU.mult,
            )
            if need_g1:
                nc.vector.tensor_mul(xlf, xlf, g1b)
            if need_b1:
                nc.vector.tensor_add(xlf, xlf, b1b)
            nc.vector.tensor_copy(xl, xlf)
        else:
            # xl = xt*rs - mu*rs in one ACT pass (per-partition scale/bias)
            nmr = lnp.tile([128, 1], F32, name="nmr", tag="nmr")
            nc.vector.tensor_scalar(
                out=nmr, in0=mv[:, 0:1], scalar1=rs, scalar2=-1.0,
                op0=ALU.mult, op1=ALU.mult,
            )
            nc.scalar.activation(xl, xt, AF.Identity, bias=nmr, scale=rs)
        # transpose via matmul with identity; one psum tile for all 4 chunks
        pt = ps_m.tile([128, DC, 128], F32, name="pt", tag="mm")
        for dc in range(DC):
            nc.tensor.matmul(
                pt[:, dc, :], lhsT=xl[:, ts(dc, 128)], rhs=ident, start=True, stop=True
            )
        # alternate the psum->sbuf copy between DVE and ACT
        if nt % 2 == 0:
            nc.vector.tensor_copy(out=xlnT[:, :, ts(nt, 128)], in_=pt)
        else:
            nc.scalar.copy(out=xlnT[:, :, ts(nt, 128)], in_=pt)

    # ================ qkv projection ================
    def emit_qk(ot, nns=(0, 1)):
        # q, k head-major: psum[o_tile, n] = sum_dc Tq[:,dc,ot].T @ xlnT[:,dc,n]
        for sec, dst in ((0, qT), (1, kT)):
            for nn in nns:
                pq = ps_m.tile([128, 512], F32, name="pq", tag="mm")
                for dc in range(DC):
                    nc.tensor.matmul(
                        pq,
                        lhsT=tq_sb[:, dc, ds(sec * INNER + ot * 128, 128)],
                        rhs=xlnT[:, dc, ts(nn, 512)],
                        start=(dc == 0), stop=(dc == DC - 1),
                    )
                nc.vector.tensor_copy(out=dst[:, ot, ts(nn, 512)], in_=pq)

    def emit_v():
        # v row-major: psum[m_tile, o] = sum_dc xlnT[:,dc,mt].T @ Tq_v[:,dc,:]
        for mt in range(NT):
            pv = ps_m.tile([128, 512], F32, name="pv", tag="mm")
            for dc in range(DC):
                nc.tensor.matmul(
                    pv,
                    lhsT=xlnT[:, dc, ts(mt, 128)],
                    rhs=tq_sb[:, dc, ds(2 * INNER, INNER)],
                    start=(dc == 0), stop=(dc == DC - 1),
                )
            nc.vector.tensor_copy(
                out=v_sb[:, mt, :, 0:DH],
                in_=pv.rearrange("p (h d) -> p h d", h=H),
            )

    # ================ scores + exp (one n-half, one head pair) ================
    def scores_exp(half, pair):
        """Returns [atn_sub0, atn_sub1], each [128, NT, 512] bf16 for this
        half's columns. Scores psums cover two m-tiles -> [128,1024] exps."""
        atns = [
            attp.tile([128, NT, NH], BF16, name=f"atn{s}", tag=f"atn{s}")
            for s in range(2)
        ]
        for mtp in range(NT // 2):
            pss = [
                ps_s.tile([128, 2, 512], F32, name="pssa", tag="s"),
                ps_s.tile([128, 2, 512], F32, name="pssb", tag="s"),
            ]
            for j in range(2):
                mt = 2 * mtp + j
                for sub in range(2):
                    base = sub * 64
                    nc.tensor.matmul(
                        pss[sub][:, j, :],
                        lhsT=kT[ds(base, 64), pair, ts(mt, 128)],
                        rhs=qT[ds(base, 64), pair, ds(half * NH, NH)],
                        start=True, stop=True,
                    )
            for sub in range(2):
                nc.scalar.activation(
                    out=atns[sub][:, 2 * mtp : 2 * mtp + 2, :], in_=pss[sub],
                    func=AF.Exp, scale=scale_exp,
                )
        return atns

    # ================ attn@v + divide for one (half, pair) ================
    def po_div(half, pair, atns):
        for sub in range(2):
            h = 2 * pair + sub
            atn = atns[sub]
            po = ps_o.tile([65, 512], F32, name="po", tag="po")
            for mt in range(NT):
                nc.tensor.matmul(
                    po,
                    lhsT=v_sb[:, mt, h, :],
                    rhs=atn[:, mt, :],
                    start=(mt == 0), stop=(mt == NT - 1),
                )
            # stage numerators + denominator row to SBUF so the psum bank
            # frees fast
            stg = smp.tile([65, 512], F32, name="stg", tag="stg")
            nc.vector.tensor_copy(stg, po)
            # even head lands on partitions 0:64 of the pair chunk directly;
            # odd head stages in aTs then partition-remaps via SBUF->SBUF DMA
            div_dst = (
                aT2[ds(0, 64), pair, ds(half * NH, NH)]
                if sub == 0
                else aTs[:, pair, ds(half * NH, NH)]
            )
            if pair < 3:
                # bounce the raw denominator row through DRAM to broadcast
                # across partitions, then approx-reciprocal (base 0 only);
                # the multiply runs on idle GPSIMD to unload DVE
                nc.sync.dma_start(out=cs_dram[h, half, :], in_=stg[64:65, :])
                rbf = smp.tile([64, 512], F32, name="rbf", tag="rbf")
                src = cs_dram[h, half, :]
                nc.sync.dma_start(
                    out=rbf,
                    in_=bass.AP(tensor=src.tensor, offset=src.offset,
                                ap=[[0, 64]] + list(src.ap)),
                )
                rcp = smp.tile([64, 512], F32, name="rcp", tag="rcp")
                nc.vector.reciprocal_approx_fast(rcp, rbf)
                nc.vector.tensor_tensor(
                    out=div_dst, in0=stg[0:DH, :], in1=rcp, op=ALU.mult,
                )
            else:
                # last pair feeds the LN2/projection tail: skip the DMA
                # round-trip. Exact reciprocal on the psum row (lane 64),
                # bf16 it, then a K=1 ones matmul broadcasts it across
                # partitions into psum; the multiply reads both psums.
                nc.vector.reciprocal(rc64f[64:65, :], po[64:65, :])
                nc.vector.tensor_copy(rc64b[64:65, :], rc64f[64:65, :])
                pb = ps_o.tile([64, 512], F32, name="pb", tag="po")
                nc.tensor.matmul(
                    pb, lhsT=ones_b64[64:65, :], rhs=rc64b[64:65, :],
                    start=True, stop=True,
                )
                nc.vector.tensor_tensor(
                    out=div_dst, in0=stg[0:DH, :], in1=pb, op=ALU.mult,
                )
            if h == 0 and half == 0:
                dump("stg_h0", stg)
                dump("atn_h0", atn)
        if pair < 3:
            nc.sync.dma_start(
                out=aT2[ds(64, 64), pair, ds(half * NH, NH)],
                in_=aTs[:, pair, ds(half * NH, NH)],
            )
            # squares for the LN2 sum-of-squares on idle GPSIMD
            nc.gpsimd.tensor_tensor(
                out=sq_sb[:, pair, ds(half * NH, NH)],
                in0=aT2[:, pair, ds(half * NH, NH)],
                in1=aT2[:, pair, ds(half * NH, NH)],
                op=ALU.mult,
            )
        else:
            # pair 3 feeds the tail: no remap — the projection reads aTs
            # directly. Squares on DVE (tail-critical).
            ev = aT2[ds(0, 64), 3, ds(half * NH, NH)]
            od = aTs[:, 3, ds(half * NH, NH)]
            nc.vector.tensor_tensor(
                out=sq_sb[ds(0, 64), 3, ds(half * NH, NH)], in0=ev, in1=ev,
                op=ALU.mult,
            )
            nc.vector.tensor_tensor(
                out=sq_lo[:, ds(half * NH, NH)], in0=od, in1=od, op=ALU.mult,
            )

    # ================ LN2 stats + scalars for one half ================
    def ln2_stats(half):
        hnt = NT // 2
        s12 = ps_o.tile([128, 2, hnt], F32, name="s12", tag="po")
        o64 = ones128[0:64, :]
        for i in range(hnt):
            nt = half * hnt + i
            for ch in range(3):
                nc.tensor.matmul(
                    s12[:, 0, i : i + 1], lhsT=aT2[:, ch, ts(nt, 128)], rhs=ones128,
                    start=(ch == 0), stop=False,
                )
            nc.tensor.matmul(
                s12[:, 0, i : i + 1], lhsT=aT2[ds(0, 64), 3, ts(nt, 128)],
                rhs=o64, start=False, stop=False,
            )
            nc.tensor.matmul(
                s12[:, 0, i : i + 1], lhsT=aTs[:, 3, ts(nt, 128)],
                rhs=o64, start=False, stop=True,
            )
            for ch in range(3):
                nc.tensor.matmul(
                    s12[:, 1, i : i + 1], lhsT=sq_sb[:, ch, ts(nt, 128)], rhs=ones128,
                    start=(ch == 0), stop=False,
                )
            nc.tensor.matmul(
                s12[:, 1, i : i + 1], lhsT=sq_sb[ds(0, 64), 3, ts(nt, 128)],
                rhs=o64, start=False, stop=False,
            )
            nc.tensor.matmul(
                s12[:, 1, i : i + 1], lhsT=sq_lo[:, ts(nt, 128)],
                rhs=o64, start=False, stop=True,
            )
        # mu = s1/512 ; var = s2/512 - mu^2 ; r2 = s_o / sqrt(var + eps_eff)
        mu, musq, var, sd2, r2, r2n = (ln2s[:, i, :] for i in range(6))
        nc.scalar.mul(mu, s12[:, 0, :], 1.0 / INNER)
        nc.vector.tensor_mul(musq, mu, mu)
        nc.vector.scalar_tensor_tensor(
            out=var, in0=s12[:, 1, :], scalar=1.0 / INNER, in1=musq,
            op0=ALU.mult, op1=ALU.subtract,
        )
        nc.scalar.activation(sd2, var, AF.Ln, bias=eps2, scale=c["inv_so2"])
        nc.scalar.activation(r2, sd2, AF.Exp, scale=-0.5)
        nc.vector.tensor_scalar_mul(r2n, r2, -1.0)

    # ================ output projection + LN2 apply for one n-tile ========
    def zy(nt):
        i = nt % (NT // 2)
        mu = ln2s[:, 0, i : i + 1]
        r2n = ln2s[:, 5, i : i + 1]
        pz = ps_m.tile([128, INNER], F32, name="pz", tag="mm")
        for ch in range(3):
            nc.tensor.matmul(
                pz, lhsT=aT2[:, ch, ts(nt, 128)], rhs=toT_sb[:, ch, :],
                start=(ch == 0), stop=False,
            )
        nc.tensor.matmul(
            pz, lhsT=aT2[ds(0, 64), 3, ts(nt, 128)], rhs=toT_sb[ds(0, 64), 3, :],
            start=False, stop=False,
        )
        nc.tensor.matmul(
            pz, lhsT=aTs[:, 3, ts(nt, 128)], rhs=toT_lo,
            start=False, stop=True,
        )
        # u = (W1*mu) - z straight off psum; y = u * (-r2) on ACT
        ut = outp.tile([128, INNER], F32, name="ut", tag="ut")
        nc.vector.scalar_tensor_tensor(
            out=ut, in0=w1b, scalar=mu, in1=pz,
            op0=ALU.mult, op1=ALU.subtract,
        )
        yt = outp.tile([128, INNER], F32, name="yt", tag="yt")
        # half-0 tiles overlap half-1 attention (DVE has slack there); the
        # final half runs after the exps, when ACT is free
        if nt < 4:
            nc.vector.tensor_scalar_mul(yt, ut, r2n)
        else:
            nc.scalar.mul(yt, ut, r2n)
        if need_bt:
            nc.vector.tensor_add(yt, yt, btb)
        q = nc.scalar if nt == NT - 1 else nc.sync
        q.dma_start(out=y[ts(nt, 128), :], in_=yt)

    # ================ driver ================
    load_weights_qk()
    load_weights_v()
    for nt in range(NT):
        phase_a(nt)
        if nt == 5:
            # first-half xlnT (tiles 0-3) is ready: start pair-0 q/k early
            emit_qk(0, nns=(0,))
    emit_qk(0, nns=(1,))
    emit_qk(1)
    a00 = scores_exp(0, 0)
    a01 = scores_exp(0, 1)
    emit_v()
    load_out_weights()
    emit_qk(2)
    po_div(0, 0, a00)
    a02 = scores_exp(0, 2)
    emit_qk(3)
    po_div(0, 1, a01)
    a03 = scores_exp(0, 3)
    po_div(0, 2, a02)
    po_div(0, 3, a03)
    dump("xlnT", xlnT)
    dump("qT", qT)
    dump("kT", kT)
    dump("v_sb", v_sb)
    dump("aT2", aT2)
    dump("sq_sb", sq_sb)
    ln2_stats(0)
    dump("ln2s", ln2s)
    # half 1 attention overlaps half 0's output projection
    a10 = scores_exp(1, 0)
    zy(0)
    zy(1)
    a11 = scores_exp(1, 1)
    po_div(1, 0, a10)
    zy(2)
    zy(3)
    a12 = scores_exp(1, 2)
    po_div(1, 1, a11)
    a13 = scores_exp(1, 3)
    po_div(1, 2, a12)
    po_div(1, 3, a13)
    ln2_stats(1)
    for nt in range(4, 8):
        zy(nt)


def _build(c: dict):
    nc = bacc.Bacc("TRN2", target_bir_lowering=False, debug=False, num_devices=B)
    io = {
        "x": nc.dram_tensor("x", [N, D], F32, kind="ExternalInput").ap(),
        "tqT": nc.dram_tensor("tqT", [D, 3 * INNER], BF16, kind="ExternalInput").ap(),
        "toT": nc.dram_tensor("toT", [INNER, INNER], BF16, kind="ExternalInput").ap(),
        "w1u": nc.dram_tensor("w1u", [INNER], F32, kind="ExternalInput").ap(),
        "y": nc.dram_tensor("y", [N, D], F32, kind="ExternalOutput").ap(),
    }
    if c["need_g1"]:
        io["g1v"] = nc.dram_tensor("g1v", [D], F32, kind="ExternalInput").ap()
    if c["need_b1"]:
        io["b1v"] = nc.dram_tensor("b1v", [D], F32, kind="ExternalInput").ap()
    if c["need_bt"]:
        io["btv"] = nc.dram_tensor("btv", [INNER], F32, kind="ExternalInput").ap()
    reps = c.get("body_reps", 1)
    with tile.TileContext(nc) as tc:
        for r in range(reps):
            with ExitStack() as ctx:
                _emit(ctx, tc, io, c, sfx="" if r == 0 else f"_r{r}")

    nc.compile()

    # The act-table-load pass greedily picks the first set containing each
    # function, thrashing between `natural_log` (Ln) and `exp_and_others`
    # (Exp) on every rstd computation. All activation funcs this kernel uses
    # (Ln, Exp, Copy, Identity) live together in `natural_log_exp_and_others`,
    # so rewrite the first load to that set and drop the rest.
    from concourse.hw_specs import get_activation_tables
    tset = list(get_activation_tables(nc.m.arch).keys())
    nle = tset.index("natural_log_exp_and_others")
    for blk in nc.main_func.blocks:
        keep, first = [], False
        for inst in blk.instructions:
            if type(inst).__name__ == "InstLoadActFuncSet":
                si = getattr(inst, "sync_info", None)
                clean = si is None or (not si.on_wait and not si.on_update)
                if not first:
                    inst.act_func_set_id = nle
                    first = True
                    keep.append(inst)
                elif not clean:
                    inst.act_func_set_id = nle
                    keep.append(inst)
            else:
                keep.append(inst)
        blk.instructions[:] = keep
    return nc


def _prep(inputs):
    g1 = np.asarray(inputs["g1"], np.float32)
    b1 = np.asarray(inputs["b1"], np.float32)
    g2 = np.asarray(inputs["g2"], np.float32)
    b2 = np.asarray(inputs["b2"], np.float32)
    b_out = np.asarray(inputs["b_out"], np.float32)

    Tq, s_q = _ternary(inputs["W_qkv"])   # [3*inner, d]
    To, s_o = _ternary(inputs["W_out"])   # [dout, o]

    Wp = To * g2[None, :]                 # fold g2 (exact when g2 == 1)
    toT = np.ascontiguousarray(Wp.T)      # [o, dout]
    w1u = Wp.sum(axis=1).astype(np.float32)
    bias_total = (b2 @ To.T) * np.float32(s_o) + b_out

    c = {
        "scale_exp": float(s_q * s_q * (DH ** -0.5)),
        "inv_so2": float(1.0 / (s_o * s_o)),
        "eps_eff": float(EPS_LN / (s_q * s_q * s_o * s_o)),
        "need_g1": bool(not np.allclose(g1, 1.0)),
        "need_b1": bool(np.any(b1)),
        "need_bt": bool(np.any(bias_total)),
    }
    arrs = {
        "tqT": np.ascontiguousarray(Tq.T),
        "toT": toT,
        "w1u": w1u,
        "g1": g1, "b1": b1, "bt": bias_total,
    }
    return c, arrs


def _to_bf16(a):
    import ml_dtypes
    return np.asarray(a, np.float32).astype(ml_dtypes.bfloat16)


def kernel(**inputs) -> np.ndarray:
    global LAST_RESULTS
    x = np.asarray(inputs["x"], np.float32)
    assert x.shape == (B, N, D)
    c, arrs = _prep(inputs)

    key = tuple(sorted(c.items()))
    if key not in _CACHE:
        _CACHE[key] = _build(c)
    nc = _CACHE[key]

    base = {
        "tqT": _to_bf16(arrs["tqT"]),
        "toT": _to_bf16(arrs["toT"]),
        "w1u": arrs["w1u"].astype(np.float32),
    }
    if c["need_g1"]:
        base["g1v"] = arrs["g1"]
    if c["need_b1"]:
        base["b1v"] = arrs["b1"]
    if c["need_bt"]:
        base["btv"] = arrs["bt"].astype(np.float32)

    in_maps = [dict(base, x=np.ascontiguousarray(x[i])) for i in range(B)]
    res = run_bass_kernel_spmd(nc, in_maps, core_ids=list(range(B)), trace=TRACE)
    LAST_RESULTS = res
    out = np.stack([res.results[i]["y"] for i in range(B)], axis=0)
    return out.astype(np.float32)


def bench_exec_ns(inputs, iters=64, reps=9, body_reps=1):
    """Measure per-execution NEFF time by chaining `iters` sequential
    executions inside one jitted program (chained through the output
    buffers) and comparing against a 1-execution program."""
    import time as _time
    import jax
    from jax.experimental.shard_map import shard_map
    from jax.sharding import Mesh, PartitionSpec, NamedSharding
    from concourse import bass2jax, mybir as _mybir

    x = np.asarray(inputs["x"], np.float32)
    c, arrs = _prep(inputs)
    if body_reps != 1:
        c["body_reps"] = body_reps
    key = tuple(sorted(c.items()))
    if key not in _CACHE:
        _CACHE[key] = _build(c)
    nc = _CACHE[key]
    bass2jax.install_neuronx_cc_hook()

    base = {
        "tqT": _to_bf16(arrs["tqT"]),
        "toT": _to_bf16(arrs["toT"]),
        "w1u": arrs["w1u"].astype(np.float32),
    }
    if c["need_g1"]:
        base["g1v"] = arrs["g1"]
    if c["need_b1"]:
        base["b1v"] = arrs["b1"]
    if c["need_bt"]:
        base["btv"] = arrs["bt"].astype(np.float32)
    in_maps = [dict(base, x=np.ascontiguousarray(x[i])) for i in range(B)]

    partition_name = nc.partition_id_tensor.name if nc.partition_id_tensor else None
    in_names, out_names, out_avals, zero_outs = [], [], [], []
    for alloc in nc.m.functions[0].allocations:
        if not isinstance(alloc, mybir.MemoryLocationSet):
            continue
        name = alloc.memorylocations[0].name
        if alloc.kind == "ExternalInput":
            if name != partition_name:
                in_names.append(name)
        elif alloc.kind == "ExternalOutput":
            out_names.append(name)
            shape = tuple(alloc.tensor_shape)
            dtype = mybir.dt.np(alloc.dtype)
            out_avals.append(jax.core.ShapedArray(shape, dtype))
            zero_outs.append(np.zeros(shape, dtype))
    n_params = len(in_names)

    bind_names = list(in_names) + list(out_names)
    if partition_name is not None:
        bind_names.append(partition_name)

    def _body(*args):
        operands = list(args)
        pid = [bass2jax.partition_id_tensor()] if partition_name else []
        outs = bass2jax._bass_exec_p.bind(
            *(operands + pid),
            out_avals=tuple(out_avals),
            in_names=tuple(bind_names),
            out_names=tuple(out_names),
            lowering_input_output_aliases=(),
            sim_require_finite=True,
            sim_require_nnan=True,
            nc=nc,
        )
        return tuple(outs)

    devices = jax.devices()[:B]
    mesh = Mesh(np.asarray(devices), ("core",))
    spec = PartitionSpec("core")
    n_out = len(out_names)
    per_core = [[np.asarray(m[nm]) for nm in in_names] for m in in_maps]
    concat_in = [
        np.concatenate([per_core[cc][i] for cc in range(B)], axis=0)
        for i in range(n_params)
    ]
    concat_zeros = [
        np.zeros((B * z.shape[0], *z.shape[1:]), z.dtype) for z in zero_outs
    ]
    dev_args = [
        jax.device_put(a, NamedSharding(mesh, spec)) for a in concat_in + concat_zeros
    ]

    f = jax.jit(
        shard_map(
            _body, mesh=mesh,
            in_specs=(spec,) * (n_params + n_out),
            out_specs=(spec,) * n_out,
            check_rep=False,
        )
    )
    jax.block_until_ready(f(*dev_args))  # compile + warm

    times = {}
    for k in (1, iters):
        best = float("inf")
        for _ in range(reps):
            t0 = _time.perf_counter()
            r = None
            for _ in range(k):
                r = f(*dev_args)  # async dispatch; device executes in-order
            jax.block_until_ready(r)
            best = min(best, _time.perf_counter() - t0)
        times[k] = best
    exec_ns = (times[iters] - times[1]) / (iters - 1) * 1e9
    return exec_ns, times


# revision 57
# speedup vs baseline: 3.0269x; 1.3958x over previous
"""Trainium2 Bass kernel for nn_Attention_6794638262338.

Single-layer attention block with BitNet-style ternary-quantized projections:
    x -> LN1 -> qkv proj (ternary W) -> MHA softmax -> LN2 -> out proj (ternary W)

Strategy: pure data parallelism. batch=8, n_cores=8 -> one batch element per
core, no collectives. Each core runs an identical Bass/Tile program.

Math folds (host side):
  - ternary_quant(W) = T * s with T in {-1,0,1}: pass T in bf16 (exact), fold
    s_qkv^2 * DIM_HEAD^-0.5 into the exp() activation scale, fold s_qkv/s_out
    into the LN2 rsqrt epsilon/scale.
  - softmax denominator: out = (sum_m exp(s)*v) / colsum. colsum obtained free
    via a ones-column in the attn@v stationary (M=65); division via DVE
    approx-reciprocal + DRAM-bounce partition broadcast + DVE multiply.
  - LN2: mean/var via ones-matmul column sums of a^T / a^T-squared; y =
    (z - mu*W1) * rstd with W1 = rowsum of effective output weight.

Schedule: n is split into two 512-column halves processed half-major, so the
output projection + LN2 of half 0 overlaps with half 1's attention. Odd heads'
attn@v psums sit at partition base 63 so divided outputs land directly on
partitions 64..127 of the pair chunk (no partition-remap DMA).
"""

import numpy as np
from contextlib import ExitStack

import concourse.bass as bass
import concourse.mybir as mybir
import concourse.tile as tile
from concourse import bacc
from concourse.bass import ts, ds
from concourse.bass_utils import run_bass_kernel_spmd
from concourse.masks import make_identity

F32 = mybir.dt.float32
BF16 = mybir.dt.bfloat16
AF = mybir.ActivationFunctionType
ALU = mybir.AluOpType

B, N, D = 8, 1024, 512
H, DH = 8, 64
INNER = H * DH  # 512
NT = N // 128   # 8 n-tiles
DC = D // 128   # 4 d-chunks
NH = N // 2     # half width (512)
EPS_LN = 1e-5
EPS_Q = 1e-6

TRACE = False          # set by test.py to capture an NTFF profile
LAST_RESULTS = None    # BassKernelResults of the most recent run

_CACHE = {}


def _ternary(w):
    """Replicate reference ternary_quant in fp32; return (unit ternary, scale)."""
    w = np.asarray(w, np.float32)
    s = np.float32(np.mean(np.abs(w), dtype=np.float32))
    t = np.round(np.clip(w / (s + np.float32(EPS_Q)), -1.0, 1.0)).astype(np.float32)
    return t, float(s)


def _emit(ctx: ExitStack, tc: "tile.TileContext", io: dict, c: dict, sfx: str = ""):
    nc = tc.nc
    dbg = c.get("debug", False)

    def dump(name, ap):
        if dbg:
            d = nc.dram_tensor(f"dbg_{name}{sfx}", list(ap.shape), ap.dtype, kind="ExternalOutput").ap()
            nc.sync.dma_start(out=d, in_=ap)
    x, tqT, toT, w1u, y = io["x"], io["tqT"], io["toT"], io["w1u"], io["y"]

    need_g1 = c["need_g1"]
    need_b1 = c["need_b1"]
    need_bt = c["need_bt"]
    scale_exp = c["scale_exp"]

    # ---------------- pools ----------------
    const_p = ctx.enter_context(tc.tile_pool(name="const" + sfx, bufs=1))
    xp = ctx.enter_context(tc.tile_pool(name="xp" + sfx, bufs=3))
    lnp = ctx.enter_context(tc.tile_pool(name="lnp" + sfx, bufs=4))
    big = ctx.enter_context(tc.tile_pool(name="big" + sfx, bufs=1))
    attp = ctx.enter_context(tc.tile_pool(name="attp" + sfx, bufs=4))
    smp = ctx.enter_context(tc.tile_pool(name="smp" + sfx, bufs=4))
    outp = ctx.enter_context(tc.tile_pool(name="outp" + sfx, bufs=2))
    # PSUM budget (8 banks): ps_s 2x[128,2,512] = 4, ps_o 2x[128,512] = 2,
    # ps_m 2x[128,512] = 2.
    ps_s = ctx.enter_context(tc.tile_pool(name="ps_s" + sfx, bufs=2, space="PSUM"))
    ps_o = ctx.enter_context(tc.tile_pool(name="ps_o" + sfx, bufs=2, space="PSUM"))
    ps_m = ctx.enter_context(tc.tile_pool(name="ps_m" + sfx, bufs=2, space="PSUM"))

    # ---------------- constants ----------------
    ident = const_p.tile([128, 128], BF16)
    make_identity(nc, ident)
    ones128 = const_p.tile([128, 1], BF16)
    nc.vector.memset(ones128, 1.0)
    # ones row at partition 64 for the K=1 reciprocal-broadcast matmul
    ones_b64 = const_p.tile([128, 64], BF16)
    nc.vector.memset(ones_b64, 1.0)
    eps1 = const_p.tile([128, 1], F32)
    nc.vector.memset(eps1, float(EPS_LN))
    eps2 = const_p.tile([128, 1], F32)
    nc.vector.memset(eps2, c["eps_eff"])
    # warm the ln/exp activation table before first use
    warm = const_p.tile([128, 1], F32)
    nc.scalar.activation(warm, eps1, AF.Ln, bias=eps1)
    nc.scalar.activation(warm, warm, AF.Exp, scale=-0.5)

    # weights arrive on the ACT DGE queue so the x tiles own the SP queue;
    # chunked + emitted late so the x tiles win the DMA engines first
    tq_sb = const_p.tile([128, DC, 3 * INNER], BF16)
    tqTr = tqT.rearrange("(c p) o -> p c o", p=128)

    def load_weights_v():
        nc.sync.dma_start(
            out=tq_sb[:, :, ds(2 * INNER, INNER)], in_=tqTr[:, :, ds(2 * INNER, INNER)]
        )

    def load_weights_qk():
        nc.sync.dma_start(
            out=tq_sb[:, :, 0 : 2 * INNER], in_=tqTr[:, :, 0 : 2 * INNER]
        )

    toT_sb = const_p.tile([128, DC, INNER], BF16)
    toT_lo = const_p.tile([64, INNER], BF16)
    w1b = const_p.tile([128, INNER], F32)

    def load_out_weights():
        toTr = toT.rearrange("(c p) o -> p c o", p=128)
        for dc in range(DC):
            nc.sync.dma_start(out=toT_sb[:, dc, :], in_=toTr[:, dc, :])
        # chunk-3 odd-head rows of the output weight, staged at partition
        # base 0 so the projection can read aTs without the partition remap
        nc.sync.dma_start(out=toT_lo, in_=toTr[ds(64, 64), 3, :])
        nc.sync.dma_start(
            out=w1b,
            in_=bass.AP(tensor=w1u.tensor, offset=w1u.offset, ap=[[0, 128]] + list(w1u.ap)),
        )
    if need_g1:
        g1_ap = io["g1v"]
        g1b = const_p.tile([128, D], F32)
        nc.gpsimd.dma_start(
            out=g1b,
            in_=bass.AP(tensor=g1_ap.tensor, offset=g1_ap.offset, ap=[[0, 128]] + list(g1_ap.ap)),
        )
    if need_b1:
        b1_ap = io["b1v"]
        b1b = const_p.tile([128, D], F32)
        nc.gpsimd.dma_start(
            out=b1b,
            in_=bass.AP(tensor=b1_ap.tensor, offset=b1_ap.offset, ap=[[0, 128]] + list(b1_ap.ap)),
        )
    if need_bt:
        bt_ap = io["btv"]
        btb = const_p.tile([128, INNER], F32)
        nc.gpsimd.dma_start(
            out=btb,
            in_=bass.AP(tensor=bt_ap.tensor, offset=bt_ap.offset, ap=[[0, 128]] + list(bt_ap.ap)),
        )

    # ---------------- persistent big tensors ----------------
    # xln^T: [d, n] bf16 as [128, DC, N]   (partition = d within chunk)
    xlnT = big.tile([128, DC, N], BF16)
    # q^T, k^T head-major: [o, n] as [128, pair, N] (o = pair*128 + p)
    qT = big.tile([128, DC, N], BF16)
    kT = big.tile([128, DC, N], BF16)
    # v row-major with ones column: [128, mt, h, 65] (m = mt*128 + p)
    v_sb = big.tile([128, NT, H, DH + 1], BF16)
    nc.vector.memset(v_sb[:, :, :, DH : DH + 1], 1.0)
    # divided attention outputs, o-major: pair chunk c = heads (2c | 2c+1)
    aT2 = big.tile([128, DC, N], BF16)
    # odd heads' divided output staging (pre partition-remap)
    aTs = big.tile([64, DC, N], BF16)
    # squares of aT2 for the LN2 sum-of-squares (+ pair-3 odd head from aTs)
    sq_sb = big.tile([128, DC, N], BF16)
    sq_lo = big.tile([64, N], BF16)
    # LN2 per-half scalar staging [mu | es | musq | var | sd2 | r2 | r2n] x 4nt
    ln2s = big.tile([128, 7, NT // 2], F32)
    # last-pair reciprocal rows (lane 64) for the PE broadcast
    rc64f = big.tile([65, 512], F32)
    rc64b = big.tile([65, 512], BF16)

    cs_dram = nc.dram_tensor("cs_scratch" + sfx, [H, 2, 512], F32).ap()

    # ================ Phase A: load x, LN1, transpose ================
    # x arrives as two individual tiles then two 3-tile strided groups:
    # tile 0 lands fast (LN1 starts ~2us) without serializing the rest
    xts = []
    for g, cnt in ((0, 1), (1, 1), (2, 3), (5, 3)):
        t = xp.tile([128, cnt, D], F32, name="xt", tag=f"xt{g}", bufs=1)
        nc.sync.dma_start(
            out=t,
            in_=bass.AP(tensor=x.tensor, offset=x.offset + g * 128 * D,
                        ap=[[D, 128], [128 * D, cnt], [1, D]]),
        )
        for i in range(cnt):
            xts.append(t[:, i, :])

    def phase_a(nt):
        xt = xts[nt]
        st6 = lnp.tile([128, 6], F32, name="st6", tag="st6")
        nc.vector.bn_stats(st6, xt)
        mv = lnp.tile([128, 2], F32, name="mv", tag="mv")
        nc.vector.bn_aggr(mv, st6)
        # rstd = exp(-0.5*ln(var+eps)) — keeps ACT on the ln/exp table set
        sd = lnp.tile([128, 1], F32, name="sd", tag="sd")
        nc.scalar.activation(sd, mv[:, 1:2], AF.Ln, bias=eps1)
        rs = lnp.tile([128, 1], F32, name="rs", tag="rs")
        nc.scalar.activation(rs, sd, AF.Exp, scale=-0.5)
        xl = xp.tile([128, D], BF16, name="xl", tag="xl")
        if need_g1 or need_b1:
            xlf = xp.tile([128, D], F32, name="xlf", tag="xlf")
            nc.vector.tensor_scalar(
                out=xlf, in0=xt, scalar1=mv[:, 0:1], scalar2=rs,
                op0=ALU.subtract, op1=ALU.mult,
            )
            if need_g1:
                nc.vector.tensor_mul(xlf, xlf, g1b)
            if need_b1:
                nc.vector.tensor_add(xlf, xlf, b1b)
            nc.vector.tensor_copy(xl, xlf)
        else:
            # xl = xt*rs - mu*rs in one ACT pass (per-partition scale/bias)
            nmr = lnp.tile([128, 1], F32, name="nmr", tag="nmr")
            nc.vector.tensor_scalar(
                out=nmr, in0=mv[:, 0:1], scalar1=rs, scalar2=-1.0,
                op0=ALU.mult, op1=ALU.mult,
            )
            nc.scalar.activation(xl, xt, AF.Identity, bias=nmr, scale=rs)
        # transpose via matmul with identity; one psum tile for all 4 chunks
        pt = ps_m.tile([128, DC, 128], F32, name="pt", tag="mm")
        for dc in range(DC):
            nc.tensor.matmul(
                pt[:, dc, :], lhsT=xl[:, ts(dc, 128)], rhs=ident, start=True, stop=True
            )
        # alternate the psum->sbuf copy between DVE and ACT
        if nt % 2 == 0:
            nc.vector.tensor_copy(out=xlnT[:, :, ts(nt, 128)], in_=pt)
        else:
            nc.scalar.copy(out=xlnT[:, :, ts(nt, 128)], in_=pt)

    # ================ qkv projection ================
    def emit_qk(ot, nns=(0, 1)):
        # q, k head-major: psum[o_tile, n] = sum_dc Tq[:,dc,ot].T @ xlnT[:,dc,n]
        for sec, dst in ((0, qT), (1, kT)):
            for nn in nns:
                pq = ps_m.tile([128, 512], F32, name="pq", tag="mm")
                for dc in range(DC):
                    nc.tensor.matmul(
                        pq,
                        lhsT=tq_sb[:, dc, ds(sec * INNER + ot * 128, 128)],
                        rhs=xlnT[:, dc, ts(nn, 512)],
                        start=(dc == 0), stop=(dc == DC - 1),
                    )
                nc.vector.tensor_copy(out=dst[:, ot, ts(nn, 512)], in_=pq)

    def emit_v(mts):
        # v row-major: psum[m_tile, o] = sum_dc xlnT[:,dc,mt].T @ Tq_v[:,dc,:]
        for mt in mts:
            pv = ps_m.tile([128, 512], F32, name="pv", tag="mm")
            for dc in range(DC):
                nc.tensor.matmul(
                    pv,
                    lhsT=xlnT[:, dc, ts(mt, 128)],
                    rhs=tq_sb[:, dc, ds(2 * INNER, INNER)],
                    start=(dc == 0), stop=(dc == DC - 1),
                )
            nc.vector.tensor_copy(
                out=v_sb[:, mt, :, 0:DH],
                in_=pv.rearrange("p (h d) -> p h d", h=H),
            )

    # ================ scores + exp (one n-half, one head pair) ================
    def scores_exp(half, pair):
        """Returns [atn_sub0, atn_sub1], each [128, NT, 512] bf16 for this
        half's columns. Scores psums cover two m-tiles -> [128,1024] exps."""
        atns = [
            attp.tile([128, NT, NH], BF16, name=f"atn{s}", tag=f"atn{s}")
            for s in range(2)
        ]
        for mtp in range(NT // 2):
            pss = [
                ps_s.tile([128, 2, 512], F32, name="pssa", tag="s"),
                ps_s.tile([128, 2, 512], F32, name="pssb", tag="s"),
            ]
            for j in range(2):
                mt = 2 * mtp + j
                for sub in range(2):
                    base = sub * 64
                    nc.tensor.matmul(
                        pss[sub][:, j, :],
                        lhsT=kT[ds(base, 64), pair, ts(mt, 128)],
                        rhs=qT[ds(base, 64), pair, ds(half * NH, NH)],
                        start=True, stop=True,
                    )
            for sub in range(2):
                nc.scalar.activation(
                    out=atns[sub][:, 2 * mtp : 2 * mtp + 2, :], in_=pss[sub],
                    func=AF.Exp, scale=scale_exp,
                )
        return atns

    # ================ attn@v + divide for one (half, pair) ================
    def po_div(half, pair, atns):
        for sub in range(2):
            h = 2 * pair + sub
            atn = atns[sub]
            po = ps_o.tile([65, 512], F32, name="po", tag="po")
            for mt in range(NT):
                nc.tensor.matmul(
                    po,
                    lhsT=v_sb[:, mt, h, :],
                    rhs=atn[:, mt, :],
                    start=(mt == 0), stop=(mt == NT - 1),
                )
            # stage numerators + denominator row to SBUF so the psum bank
            # frees fast
            stg = smp.tile([65, 512], F32, name="stg", tag="stg")
            nc.vector.tensor_copy(stg, po)
            # even head lands on partitions 0:64 of the pair chunk directly;
            # odd head stages in aTs then partition-remaps via SBUF->SBUF DMA
            div_dst = (
                aT2[ds(0, 64), pair, ds(half * NH, NH)]
                if sub == 0
                else aTs[:, pair, ds(half * NH, NH)]
            )
            if pair < 3:
                # bounce the raw denominator row through DRAM to broadcast
                # across partitions, then approx-reciprocal (base 0 only);
                # the multiply runs on idle GPSIMD to unload DVE
                nc.sync.dma_start(out=cs_dram[h, half, :], in_=stg[64:65, :])
                rbf = smp.tile([64, 512], F32, name="rbf", tag="rbf")
                src = cs_dram[h, half, :]
                nc.sync.dma_start(
                    out=rbf,
                    in_=bass.AP(tensor=src.tensor, offset=src.offset,
                                ap=[[0, 64]] + list(src.ap)),
                )
                rcp = smp.tile([64, 512], F32, name="rcp", tag="rcp")
                nc.vector.reciprocal_approx_fast(rcp, rbf)
                nc.vector.tensor_tensor(
                    out=div_dst, in0=stg[0:DH, :], in1=rcp, op=ALU.mult,
                )
            else:
                # last pair feeds the LN2/projection tail: skip the DMA
                # round-trip. Exact reciprocal on the psum row (lane 64),
                # bf16 it, then a K=1 ones matmul broadcasts it across
                # partitions into psum; the multiply reads both psums.
                nc.vector.reciprocal(rc64f[64:65, :], po[64:65, :])
                nc.vector.tensor_copy(rc64b[64:65, :], rc64f[64:65, :])
                pb = ps_o.tile([64, 512], F32, name="pb", tag="po")
                nc.tensor.matmul(
                    pb, lhsT=ones_b64[64:65, :], rhs=rc64b[64:65, :],
                    start=True, stop=True,
                )
                nc.vector.tensor_tensor(
                    out=div_dst, in0=stg[0:DH, :], in1=pb, op=ALU.mult,
                )
            if h == 0 and half == 0:
                dump("stg_h0", stg)
                dump("atn_h0", atn)
        if pair < 3:
            nc.sync.dma_start(
                out=aT2[ds(64, 64), pair, ds(half * NH, NH)],
                in_=aTs[:, pair, ds(half * NH, NH)],
            )
            # squares for the LN2 sum-of-squares on idle GPSIMD
            nc.gpsimd.tensor_tensor(
                out=sq_sb[:, pair, ds(half * NH, NH)],
                in0=aT2[:, pair, ds(half * NH, NH)],
                in1=aT2[:, pair, ds(half * NH, NH)],
                op=ALU.mult,
            )
        else:
            # pair 3 feeds the tail: no remap — the projection reads aTs
            # directly. Squares on DVE (tail-critical).
            ev = aT2[ds(0, 64), 3, ds(half * NH, NH)]
            od = aTs[:, 3, ds(half * NH, NH)]
            nc.vector.tensor_tensor(
                out=sq_sb[ds(0, 64), 3, ds(half * NH, NH)], in0=ev, in1=ev,
                op=ALU.mult,
            )
            nc.vector.tensor_tensor(
                out=sq_lo[:, ds(half * NH, NH)], in0=od, in1=od, op=ALU.mult,
            )

    # ================ LN2 stats + scalars for one half ================
    def ln2_stats(half):
        hnt = NT // 2
        s12 = ps_o.tile([128, 2, hnt], F32, name="s12", tag="po")
        o64 = ones128[0:64, :]
        for i in range(hnt):
            nt = half * hnt + i
            for ch in range(3):
                nc.tensor.matmul(
                    s12[:, 0, i : i + 1], lhsT=aT2[:, ch, ts(nt, 128)], rhs=ones128,
                    start=(ch == 0), stop=False,
                )
            nc.tensor.matmul(
                s12[:, 0, i : i + 1], lhsT=aT2[ds(0, 64), 3, ts(nt, 128)],
                rhs=o64, start=False, stop=False,
            )
            nc.tensor.matmul(
                s12[:, 0, i : i + 1], lhsT=aTs[:, 3, ts(nt, 128)],
                rhs=o64, start=False, stop=True,
            )
            for ch in range(3):
                nc.tensor.matmul(
                    s12[:, 1, i : i + 1], lhsT=sq_sb[:, ch, ts(nt, 128)], rhs=ones128,
                    start=(ch == 0), stop=False,
                )
            nc.tensor.matmul(
                s12[:, 1, i : i + 1], lhsT=sq_sb[ds(0, 64), 3, ts(nt, 128)],
                rhs=o64, start=False, stop=False,
            )
            nc.tensor.matmul(
                s12[:, 1, i : i + 1], lhsT=sq_lo[:, ts(nt, 128)],
                rhs=o64, start=False, stop=True,
            )
        # mu = s1/512 ; var = s2/512 - mu^2 ; r2 = s_o / sqrt(var + eps_eff)
        mu, musq, var, sd2, r2, r2n = (ln2s[:, i, :] for i in range(6))
        nc.scalar.mul(mu, s12[:, 0, :], 1.0 / INNER)
        nc.vector.tensor_mul(musq, mu, mu)
        nc.vector.scalar_tensor_tensor(
            out=var, in0=s12[:, 1, :], scalar=1.0 / INNER, in1=musq,
            op0=ALU.mult, op1=ALU.subtract,
        )
        nc.scalar.activation(sd2, var, AF.Ln, bias=eps2, scale=c["inv_so2"])
        nc.scalar.activation(r2, sd2, AF.Exp, scale=-0.5)
        nc.vector.tensor_scalar_mul(r2n, r2, -1.0)

    # ================ output projection + LN2 apply for one n-tile ========
    def zy_mm(nt):
        pz = ps_m.tile([128, INNER], F32, name="pz", tag="mm")
        for ch in range(3):
            nc.tensor.matmul(
                pz, lhsT=aT2[:, ch, ts(nt, 128)], rhs=toT_sb[:, ch, :],
                start=(ch == 0), stop=False,
            )
        nc.tensor.matmul(
            pz, lhsT=aT2[ds(0, 64), 3, ts(nt, 128)], rhs=toT_sb[ds(0, 64), 3, :],
            start=False, stop=False,
        )
        nc.tensor.matmul(
            pz, lhsT=aTs[:, 3, ts(nt, 128)], rhs=toT_lo,
            start=False, stop=True,
        )
        return pz

    def zy(nt, pz=None):
        i = nt % (NT // 2)
        mu = ln2s[:, 0, i : i + 1]
        r2n = ln2s[:, 5, i : i + 1]
        if pz is None:
            pz = zy_mm(nt)
        # u = (W1*mu) - z straight off psum; y = u * (-r2) on ACT
        ut = outp.tile([128, INNER], F32, name="ut", tag="ut")
        nc.vector.scalar_tensor_tensor(
            out=ut, in0=w1b, scalar=mu, in1=pz,
            op0=ALU.mult, op1=ALU.subtract,
        )
        yt = outp.tile([128, INNER], F32, name="yt", tag="yt")
        # half-0 tiles overlap half-1 attention (DVE has slack there); the
        # final half runs after the exps, when ACT is free
        if nt < 4:
            nc.vector.tensor_scalar_mul(yt, ut, r2n)
        else:
            nc.scalar.mul(yt, ut, r2n)
        if need_bt:
            nc.vector.tensor_add(yt, yt, btb)
        q = nc.scalar if nt == NT - 1 else nc.sync
        q.dma_start(out=y[ts(nt, 128), :], in_=yt)

    # ================ driver ================
    load_weights_qk()
    load_weights_v()
    for nt in range(NT):
        phase_a(nt)
        if nt == 5:
            # first-half xlnT (tiles 0-3) is ready: start pair-0 q/k early
            emit_qk(0, nns=(0,))
    emit_qk(0, nns=(1,))
    a00 = scores_exp(0, 0)
    emit_qk(1)
    emit_v(range(0, 4))
    a01 = scores_exp(0, 1)
    emit_v(range(4, 8))
    load_out_weights()
    emit_qk(2)
    po_div(0, 0, a00)
    a02 = scores_exp(0, 2)
    emit_qk(3)
    po_div(0, 1, a01)
    a03 = scores_exp(0, 3)
    po_div(0, 2, a02)
    po_div(0, 3, a03)
    dump("xlnT", xlnT)
    dump("qT", qT)
    dump("kT", kT)
    dump("v_sb", v_sb)
    dump("aT2", aT2)
    dump("sq_sb", sq_sb)
    ln2_stats(0)
    dump("ln2s", ln2s)
    # half 1 attention overlaps half 0's output projection
    a10 = scores_exp(1, 0)
    zy(0)
    zy(1)
    a11 = scores_exp(1, 1)
    po_div(1, 0, a10)
    zy(2)
    zy(3)
    a12 = scores_exp(1, 2)
    po_div(1, 1, a11)
    a13 = scores_exp(1, 3)
    po_div(1, 2, a12)
    po_div(1, 3, a13)
    # the first two projections' matmuls only need the divided values, so
    # they issue ahead of the LN2 stats chain
    pz4 = zy_mm(4)
    pz5 = zy_mm(5)
    ln2_stats(1)
    zy(4, pz4)
    zy(5, pz5)
    zy(6)
    zy(7)


def _build(c: dict):
    nc = bacc.Bacc("TRN2", target_bir_lowering=False, debug=False, num_devices=B)
    io = {
        "x": nc.dram_tensor("x", [N, D], F32, kind="ExternalInput").ap(),
        "tqT": nc.dram_tensor("tqT", [D, 3 * INNER], BF16, kind="ExternalInput").ap(),
        "toT": nc.dram_tensor("toT", [INNER, INNER], BF16, kind="ExternalInput").ap(),
        "w1u": nc.dram_tensor("w1u", [INNER], F32, kind="ExternalInput").ap(),
        "y": nc.dram_tensor("y", [N, D], F32, kind="ExternalOutput").ap(),
    }
    if c["need_g1"]:
        io["g1v"] = nc.dram_tensor("g1v", [D], F32, kind="ExternalInput").ap()
    if c["need_b1"]:
        io["b1v"] = nc.dram_tensor("b1v", [D], F32, kind="ExternalInput").ap()
    if c["need_bt"]:
        io["btv"] = nc.dram_tensor("btv", [INNER], F32, kind="ExternalInput").ap()
    reps = c.get("body_reps", 1)
    with tile.TileContext(nc) as tc:
        for r in range(reps):
            with ExitStack() as ctx:
                _emit(ctx, tc, io, c, sfx="" if r == 0 else f"_r{r}")

    nc.compile()

    # The act-table-load pass greedily picks the first set containing each
    # function, thrashing between `natural_log` (Ln) and `exp_and_others`
    # (Exp) on every rstd computation. All activation funcs this kernel uses
    # (Ln, Exp, Copy, Identity) live together in `natural_log_exp_and_others`,
    # so rewrite the first load to that set and drop the rest.
    from concourse.hw_specs import get_activation_tables
    tset = list(get_activation_tables(nc.m.arch).keys())
    nle = tset.index("natural_log_exp_and_others")
    for blk in nc.main_func.blocks:
        keep, first = [], False
        for inst in blk.instructions:
            if type(inst).__name__ == "InstLoadActFuncSet":
                si = getattr(inst, "sync_info", None)
                clean = si is None or (not si.on_wait and not si.on_update)
                if not first:
                    inst.act_func_set_id = nle
                    first = True
                    keep.append(inst)
                elif not clean:
                    inst.act_func_set_id = nle
                    keep.append(inst)
            else:
                keep.append(inst)
        blk.instructions[:] = keep
    return nc


def _prep(inputs):
    g1 = np.asarray(inputs["g1"], np.float32)
    b1 = np.asarray(inputs["b1"], np.float32)
    g2 = np.asarray(inputs["g2"], np.float32)
    b2 = np.asarray(inputs["b2"], np.float32)
    b_out = np.asarray(inputs["b_out"], np.float32)

    Tq, s_q = _ternary(inputs["W_qkv"])   # [3*inner, d]
    To, s_o = _ternary(inputs["W_out"])   # [dout, o]

    Wp = To * g2[None, :]                 # fold g2 (exact when g2 == 1)
    toT = np.ascontiguousarray(Wp.T)      # [o, dout]
    w1u = Wp.sum(axis=1).astype(np.float32)
    bias_total = (b2 @ To.T) * np.float32(s_o) + b_out

    c = {
        "scale_exp": float(s_q * s_q * (DH ** -0.5)),
        "inv_so2": float(1.0 / (s_o * s_o)),
        "eps_eff": float(EPS_LN / (s_q * s_q * s_o * s_o)),
        "need_g1": bool(not np.allclose(g1, 1.0)),
        "need_b1": bool(np.any(b1)),
        "need_bt": bool(np.any(bias_total)),
    }
    arrs = {
        "tqT": np.ascontiguousarray(Tq.T),
        "toT": toT,
        "w1u": w1u,
        "g1": g1, "b1": b1, "bt": bias_total,
    }
    return c, arrs


def _to_bf16(a):
    import ml_dtypes
    return np.asarray(a, np.float32).astype(ml_dtypes.bfloat16)


def kernel(**inputs) -> np.ndarray:
    global LAST_RESULTS
    x = np.asarray(inputs["x"], np.float32)
    assert x.shape == (B, N, D)
    c, arrs = _prep(inputs)

    key = tuple(sorted(c.items()))
    if key not in _CACHE:
        _CACHE[key] = _build(c)
    nc = _CACHE[key]

    base = {
        "tqT": _to_bf16(arrs["tqT"]),
        "toT": _to_bf16(arrs["toT"]),
        "w1u": arrs["w1u"].astype(np.float32),
    }
    if c["need_g1"]:
        base["g1v"] = arrs["g1"]
    if c["need_b1"]:
        base["b1v"] = arrs["b1"]
    if c["need_bt"]:
        base["btv"] = arrs["bt"].astype(np.float32)

    in_maps = [dict(base, x=np.ascontiguousarray(x[i])) for i in range(B)]
    res = run_bass_kernel_spmd(nc, in_maps, core_ids=list(range(B)), trace=TRACE)
    LAST_RESULTS = res
    out = np.stack([res.results[i]["y"] for i in range(B)], axis=0)
    return out.astype(np.float32)


def bench_exec_ns(inputs, iters=64, reps=9, body_reps=1):
    """Measure per-execution NEFF time by chaining `iters` sequential
    executions inside one jitted program (chained through the output
    buffers) and comparing against a 1-execution program."""
    import time as _time
    import jax
    from jax.experimental.shard_map import shard_map
    from jax.sharding import Mesh, PartitionSpec, NamedSharding
    from concourse import bass2jax, mybir as _mybir

    x = np.asarray(inputs["x"], np.float32)
    c, arrs = _prep(inputs)
    if body_reps != 1:
        c["body_reps"] = body_reps
    key = tuple(sorted(c.items()))
    if key not in _CACHE:
        _CACHE[key] = _build(c)
    nc = _CACHE[key]
    bass2jax.install_neuronx_cc_hook()

    base = {
        "tqT": _to_bf16(arrs["tqT"]),
        "toT": _to_bf16(arrs["toT"]),
        "w1u": arrs["w1u"].astype(np.float32),
    }
    if c["need_g1"]:
        base["g1v"] = arrs["g1"]
    if c["need_b1"]:
        base["b1v"] = arrs["b1"]
    if c["need_bt"]:
        base["btv"] = arrs["bt"].astype(np.float32)
    in_maps = [dict(base, x=np.ascontiguousarray(x[i])) for i in range(B)]

    partition_name = nc.partition_id_tensor.name if nc.partition_id_tensor else None
    in_names, out_names, out_avals, zero_outs = [], [], [], []
    for alloc in nc.m.functions[0].allocations:
        if not isinstance(alloc, mybir.MemoryLocationSet):
            continue
        name = alloc.memorylocations[0].name
        if alloc.kind == "ExternalInput":
            if name != partition_name:
                in_names.append(name)
        elif alloc.kind == "ExternalOutput":
            out_names.append(name)
            shape = tuple(alloc.tensor_shape)
            dtype = mybir.dt.np(alloc.dtype)
            out_avals.append(jax.core.ShapedArray(shape, dtype))
            zero_outs.append(np.zeros(shape, dtype))
    n_params = len(in_names)

    bind_names = list(in_names) + list(out_names)
    if partition_name is not None:
        bind_names.append(partition_name)

    def _body(*args):
        operands = list(args)
        pid = [bass2jax.partition_id_tensor()] if partition_name else []
        outs = bass2jax._bass_exec_p.bind(
            *(operands + pid),
            out_avals=tuple(out_avals),
            in_names=tuple(bind_names),
            out_names=tuple(out_names),
            lowering_input_output_aliases=(),
            sim_require_finite=True,
            sim_require_nnan=True,
            nc=nc,
        )
        return tuple(outs)

    devices = jax.devices()[:B]
    mesh = Mesh(np.asarray(devices), ("core",))
    spec = PartitionSpec("core")
    n_out = len(out_names)
    per_core = [[np.asarray(m[nm]) for nm in in_names] for m in in_maps]
    concat_in = [
        np.concatenate([per_core[cc][i] for cc in range(B)], axis=0)
        for i in range(n_params)
    ]
    concat_zeros = [
        np.zeros((B * z.shape[0], *z.shape[1:]), z.dtype) for z in zero_outs
    ]
    dev_args = [
        jax.device_put(a, NamedSharding(mesh, spec)) for a in concat_in + concat_zeros
    ]

    f = jax.jit(
        shard_map(
            _body, mesh=mesh,
            in_specs=(spec,) * (n_params + n_out),
            out_specs=(spec,) * n_out,
            check_rep=False,
        )
    )
    jax.block_until_ready(f(*dev_args))  # compile + warm

    times = {}
    for k in (1, iters):
        best = float("inf")
        for _ in range(reps):
            t0 = _time.perf_counter()
            r = None
            for _ in range(k):
                r = f(*dev_args)  # async dispatch; device executes in-order
            jax.block_until_ready(r)
            best = min(best, _time.perf_counter() - t0)
        times[k] = best
    exec_ns = (times[iters] - times[1]) / (iters - 1) * 1e9
    return exec_ns, times
